# revision 9
# baseline (speedup 1.0000x reference)
"""Self-contained 2-layer GAT kernel for Trainium2, 8-core SPMD. v2.

Edge-major, bf16 data path. Per core, per layer:

  node phase:  [h | aS | aD] = xtb_chunk.T @ wcatb  (bf16 PE, fp32 PSUM)
      -> qtab rows [h|aS] (256 bf16 = 512B, gatherable) in DRAM
      -> adtab rows [aD|pad] (64 f32 = 256B) for OWN dst nodes only.
      xtb is rotated per core so own node chunks are at positions 0..NT-1
      (SPMD: all cores run identical code; the rotation lives in the data).

  edge phase (per group of G dst tiles, edges padded to 128-chunks):
      dma_gather q rows by permuted src id (int16, lo/hi half tables;
          wrong-half edges land on a zero row)                       1+1 ops
      dma_gather aD rows from the group window of adtab (idx < G*128) 1 op
      alpha = lrelu(aS + aD); ex = exp(alpha) -> msgex[:, F:FA]      (DVE+ACT)
      Ind one-hot bf16 per chunk                                     (DVE+POOL)
      msgex[:, 0:F] = h * ex broadcast (bf16)                        (DVE)
      per tile: PSUM accum over its chunks: [out|den] += Ind.T @ msgex  (PE)
      epilogue: relu(out/den + b) -> out rows.
"""

import os
import sys
import numpy as np

sys.path.insert(0, "/opt/trn_rl_repo")

import concourse.bacc as bacc
import concourse.mybir as mybir
from concourse.bass_utils import run_bass_kernel_spmd
from concourse.tile import TileContext

f32 = mybir.dt.float32
bf16 = mybir.dt.bfloat16
i16 = mybir.dt.int16
i32 = mybir.dt.int32

P = 128
H = 4
C = 32
F = 128          # feature width (= H*C)
FA = F + H       # h | aS used columns
FW = F + 2 * H   # node-phase matmul width: h | aS | aD
QSTEP = 256      # qtab row stride, bf16 elems (512B rows)
QPAY = 256       # gathered payload per row (bf16 elems)
ASTEP = 64       # adtab row stride, f32 elems (256B)
APAY = 64        # gathered payload (f32 elems)
G = 2            # dst tiles per gather group
NSWQ = 4         # SWDGE queues; gathers round-robin across them

N_CORES = 8
N_NODES = 50000
N_EDGES = 800000

_SKIP = set(os.environ.get("GAT_SKIP", "").split(","))


def _f32_to_bf16_bits(x):
    """Round-to-nearest-even fp32 -> bf16 (ml_dtypes.bfloat16 array)."""
    import ml_dtypes
    u = np.asarray(x, np.float32).view(np.uint32)
    r = ((u + 0x7FFF + ((u >> 16) & 1)) >> 16).astype(np.uint16)
    return r.view(ml_dtypes.bfloat16)


def _make_plan(src, dst, N, n_cores):
    npad = ((N + P * n_cores - 1) // (P * n_cores)) * (P * n_cores)
    npc = npad // n_cores
    NT = npc // P                  # dst tiles per core
    NCH = npad // P                # total node chunks
    NCHH = NCH // 2
    NLO = npad // 2                # permuted ids in the lo half
    L = NLO + P                    # half-table rows (incl. one zero chunk)
    NROW = 2 * L
    assert L <= 32767 and NLO % P == 0
    ZLO = NLO                      # half-local zero-row id (both halves)
    NG = (NT + G - 1) // G         # gather groups per core

    # per-core permutation: global chunk gch sits at position (gch - c*NT) % NCH
    # bucket edges per (core, group, tile, half)
    core_of = dst // npc
    tile_of = (dst % npc) // P
    dstloc = dst % P

    cnt = np.zeros((n_cores, NT, 2), np.int64)
    pq = np.empty((n_cores, len(src)), np.int64)    # permuted position ids
    phalf = np.empty((n_cores, len(src)), np.int64)
    for c in range(n_cores):
        pch = (src // P - c * NT) % NCH
        q = pch * P + (src % P)
        pq[c] = np.where(q >= NLO, q - NLO, q)      # half-local row
        phalf[c] = (q >= NLO).astype(np.int64)
        m = core_of == c
        np.add.at(cnt, (np.full(m.sum(), c), tile_of[m], phalf[c][m]), 1)

    Klo = np.maximum(1, np.ceil(cnt[:, :, 0].max(axis=0) / P).astype(np.int64))
    Khi = np.ceil(cnt[:, :, 1].max(axis=0) / P).astype(np.int64)
    K = Klo + Khi

    # flat chunk order: per group: [lo chunks t asc][hi chunks t asc]
    chunk_tile = []          # tile of each flat chunk
    chunks_lo = {}           # tile -> (start, count) in flat order
    chunks_hi = {}
    group_span = []          # (flat_start, n_lo_chunks, n_hi_chunks) per group
    pos = 0
    for g in range(NG):
        tiles = range(g * G, min(NT, (g + 1) * G))
        g0 = pos
        for t in tiles:
            chunks_lo[t] = (pos, int(Klo[t]))
            pos += int(Klo[t])
            chunk_tile += [t] * int(Klo[t])
        nlo = pos - g0
        for t in tiles:
            chunks_hi[t] = (pos, int(Khi[t]))
            pos += int(Khi[t])
            chunk_tile += [t] * int(Khi[t])
        group_span.append((g0, nlo, pos - g0 - nlo))
    TOTCH = pos

    gsrci = np.full((n_cores, 16, 8 * TOTCH), ZLO, np.int16)
    gdwin = np.zeros((n_cores, 16, 8 * TOTCH), np.int16)
    gloc = np.full((n_cores, P, TOTCH), float(P), np.float32)

    eorder = np.argsort(core_of * NT + tile_of, kind="stable")
    tstarts = np.searchsorted((core_of * NT + tile_of)[eorder],
                              np.arange(n_cores * NT))
    tends = np.searchsorted((core_of * NT + tile_of)[eorder],
                            np.arange(n_cores * NT) + 1)

    for c in range(n_cores):
        for t in range(NT):
            ev = eorder[tstarts[c * NT + t]:tends[c * NT + t]]
            hb = phalf[c][ev]
            u = t % G
            for half, (k0, nk) in ((0, chunks_lo[t]), (1, chunks_hi[t])):
                if nk == 0:
                    continue
                sel = ev[hb == half]
                n = len(sel)
                npadn = nk * P
                sv = np.full(npadn, ZLO, np.int64)
                lv = np.full(npadn, P, np.int64)
                wv = np.full(npadn, u * P, np.int64)
                if n:
                    sv[:n] = pq[c][sel]
                    lv[:n] = dstloc[sel]
                    wv[:n] = u * P + dstloc[sel]
                j = np.arange(npadn)
                cc = 8 * k0 + j // 16
                rr = j % 16
                gsrci[c, rr, cc] = sv
                gdwin[c, rr, cc] = wv
                gloc[c, j % P, k0 + j // P] = lv

    gsrci = np.tile(gsrci, (1, 8, 1))
    gdwin = np.tile(gdwin, (1, 8, 1))

    return dict(
        n_cores=n_cores, N=N, npad=npad, npc=npc, NT=NT, NCH=NCH, NCHH=NCHH,
        NLO=NLO, L=L, NROW=NROW, NG=NG,
        K=[int(k) for k in K], Klo=[int(k) for k in Klo],
        TOTCH=TOTCH, chunk_tile=chunk_tile,
        chunks_lo=chunks_lo, chunks_hi=chunks_hi, group_span=group_span,
        gsrci=gsrci, gdwin=gdwin, gloc=gloc,
    )


def _layer_inputs(plan, x, W, a_src, a_dst, b):
    npad, NCH, NT = plan["npad"], plan["NCH"], plan["NT"]
    n_cores = plan["n_cores"]
    xp = np.zeros((npad, F), dtype=np.float32)
    xp[: x.shape[0]] = x
    xb = _f32_to_bf16_bits(xp).reshape(NCH, P, F)
    Ablk_s = np.zeros((F, H), dtype=np.float32)
    Ablk_d = np.zeros((F, H), dtype=np.float32)
    for h in range(H):
        Ablk_s[h * C:(h + 1) * C, h] = a_src[h]
        Ablk_d[h * C:(h + 1) * C, h] = a_dst[h]
    W = np.asarray(W, dtype=np.float32)
    wcat = np.concatenate([W, W @ Ablk_s, W @ Ablk_d], axis=1)
    wcatb = _f32_to_bf16_bits(wcat)
    brep = np.broadcast_to(np.asarray(b, np.float32), (P, F)).copy()
    maps = []
    for c in range(n_cores):
        perm = (np.arange(NCH) + c * NT) % NCH      # position p <- chunk perm[p]
        xtc = np.ascontiguousarray(
            xb[perm].transpose(0, 2, 1))            # [NCH, F, P] uint16
        maps.append(dict(
            xtb=xtc, wcatb=wcatb, brep=brep,
            gsrci=plan["gsrci"][c], gdwin=plan["gdwin"][c],
            gloc=plan["gloc"][c]))
    return maps


def _build_layer_kernel(plan):
    NT, NCH, TOTCH = plan["NT"], plan["NCH"], plan["TOTCH"]
    NCHH, NG = plan["NCHH"], plan["NG"]
    K, Klo = plan["K"], plan["Klo"]
    L, NROW, NLO = plan["L"], plan["NROW"], plan["NLO"]
    chunk_tile = plan["chunk_tile"]
    chunks_lo, chunks_hi = plan["chunks_lo"], plan["chunks_hi"]
    group_span = plan["group_span"]

    nc = bacc.Bacc(num_swdge_queues=NSWQ)
    xtb = nc.dram_tensor("xtb", [NCH, F, P], bf16, kind="ExternalInput")
    wcatb = nc.dram_tensor("wcatb", [F, FW], bf16, kind="ExternalInput")
    brep = nc.dram_tensor("brep", [P, F], f32, kind="ExternalInput")
    gsrci = nc.dram_tensor("gsrci", [P, 8 * TOTCH], i16, kind="ExternalInput")
    gdwin = nc.dram_tensor("gdwin", [P, 8 * TOTCH], i16, kind="ExternalInput")
    gloc = nc.dram_tensor("gloc", [P, TOTCH], f32, kind="ExternalInput")
    out = nc.dram_tensor("out", [NT * P, F], f32, kind="ExternalOutput")

    qtab = nc.dram_tensor("qtab", [NROW, QSTEP], bf16)
    adtab = nc.dram_tensor("adtab", [NT * P, ASTEP], f32)

    def rowbase(g):
        return g * P if g < NCHH else g * P + P

    # ---- Phase 1: node phase ----
    with TileContext(nc) as tc:
        with (
            tc.tile_pool(name="const", bufs=1) as cpool,
            tc.tile_pool(name="nodein", bufs=4) as npool,
            tc.tile_pool(name="nodeout", bufs=4) as hpool,
            tc.tile_pool(name="adout", bufs=4) as adpool,
            tc.tile_pool(name="npsum", bufs=4, space="PSUM") as npsum,
        ):
            wcat_sb = cpool.tile([F, FW], bf16)
            nc.sync.dma_start(wcat_sb[:, :], wcatb[:, :])

            zt = cpool.tile([P, QSTEP], bf16)
            nc.vector.memset(zt[:, :], 0.0)
            for zr in (NLO, NROW - P):
                nc.sync.dma_start(qtab[zr:zr + P, :], zt[:, :])

            NB = 28
            node_batches = [] if "node" in _SKIP else [
                (bb, min(NB, NCHH - (bb % NCHH)))
                for bb in list(range(0, NCHH, NB)) + list(range(NCHH, NCH, NB))
            ]
            for bb, nb in node_batches:
                rb = rowbase(bb)
                xcb = npool.tile([F, NB, P], bf16, tag="xc")
                nc.sync.dma_start(
                    xcb[:, 0:nb, :],
                    xtb[bb:bb + nb, :, :].rearrange("n f p -> f n p"))
                hcb = hpool.tile([P, NB, FA], bf16, tag="hc")
                adcb = adpool.tile([P, NB, H], f32, tag="adc")
                for k in range(nb):
                    ps = npsum.tile([P, FW], f32, tag="nps")
                    nc.tensor.matmul(ps[:, :], lhsT=xcb[:, k, :],
                                     rhs=wcat_sb[:, :], start=True, stop=True)
                    if k % 2 == 0:
                        nc.scalar.activation(
                            hcb[:, k, :], ps[:, 0:FA],
                            mybir.ActivationFunctionType.Copy)
                    else:
                        nc.vector.tensor_copy(hcb[:, k, :], ps[:, 0:FA])
                    if bb + nb <= NT or bb < NT:
                        if bb + k < NT:
                            nc.vector.tensor_copy(adcb[:, k, :],
                                                  ps[:, FA:FW])
                nc.sync.dma_start(
                    qtab[rb:rb + nb * P, 0:FA].rearrange(
                        "(n p) w -> p n w", p=P),
                    hcb[:, 0:nb, :])
                if bb < NT:
                    na = min(nb, NT - bb)
                    nc.sync.dma_start(
                        adtab[bb * P:(bb + na) * P, 0:H].rearrange(
                            "(n p) w -> p n w", p=P),
                        adcb[:, 0:na, :])

    # ---- Phase 2: edge phase ----
    with TileContext(nc) as tc:
        with (
            tc.tile_pool(name="econst", bufs=1) as cpool,
            tc.tile_pool(name="egather", bufs=2) as gpool,
            tc.tile_pool(name="eind", bufs=2) as ipool,
            tc.tile_pool(name="ealpha", bufs=2) as apool,
            tc.tile_pool(name="emsg", bufs=2) as mpool,
            tc.tile_pool(name="epsum", bufs=4, space="PSUM") as epsum,
            tc.tile_pool(name="eout", bufs=3) as opool,
        ):
            iota_i = cpool.tile([P, P], i32)
            nc.gpsimd.iota(iota_i[:, :], pattern=[[1, P]], base=0,
                           channel_multiplier=0)
            iota_b = cpool.tile([P, P], bf16)
            nc.vector.tensor_copy(iota_b[:, :], iota_i[:, :])
            brep_sb = cpool.tile([P, F], f32)
            nc.sync.dma_start(brep_sb[:, :], brep[:, :])

            srcA = cpool.tile([P, 8 * TOTCH], i16)
            nc.sync.dma_start(srcA[:, :], gsrci[:, :])
            dwinA = cpool.tile([P, 8 * TOTCH], i16)
            nc.sync.dma_start(dwinA[:, :], gdwin[:, :])
            locA = cpool.tile([P, TOTCH], f32)
            nc.sync.dma_start(locA[:, :], gloc[:, :])

            for g in range(NG):
                tiles = list(range(g * G, min(NT, (g + 1) * G)))
                g0, nglo, nghi = group_span[g]
                Kg = nglo + nghi

                hsa = gpool.tile([P, Kg, QPAY], bf16, tag="hsa")
                if "hgather" not in _SKIP:
                    nc.gpsimd.dma_gather(
                        out_ap=hsa[:, 0:nglo, :], in_ap=qtab[0:L, 0:QPAY],
                        idxs_ap=srcA[:, 8 * g0:8 * (g0 + nglo)],
                        num_idxs=nglo * P, num_idxs_reg=nglo * P,
                        elem_size=QPAY, elem_step=QSTEP, single_packet=False,
                        queue_num=(3 * g) % NSWQ)
                    if nghi > 0:
                        nc.gpsimd.dma_gather(
                            out_ap=hsa[:, nglo:Kg, :],
                            in_ap=qtab[L:NROW, 0:QPAY],
                            idxs_ap=srcA[:, 8 * (g0 + nglo):8 * (g0 + Kg)],
                            num_idxs=nghi * P, num_idxs_reg=nghi * P,
                            elem_size=QPAY, elem_step=QSTEP,
                            single_packet=False,
                            queue_num=(3 * g + 1) % NSWQ)
                else:
                    nc.vector.memset(hsa[:, :, 0:1], 0.0)

                adw = gpool.tile([P, Kg, APAY], f32, tag="adw")
                if "adgather" not in _SKIP:
                    nc.gpsimd.dma_gather(
                        out_ap=adw[:, :, :],
                        in_ap=adtab[g * G * P:(g * G + len(tiles)) * P,
                                    0:APAY],
                        idxs_ap=dwinA[:, 8 * g0:8 * (g0 + Kg)],
                        num_idxs=Kg * P, num_idxs_reg=Kg * P,
                        elem_size=APAY, elem_step=ASTEP,
                        single_packet=False,
                        queue_num=(3 * g + 2) % NSWQ)
                else:
                    nc.vector.memset(adw[:, :, 0:1], 0.0)

                ind = ipool.tile([P, Kg, P], bf16, tag="ind")
                if "ind" not in _SKIP:
                    for k in range(Kg):
                        eng = nc.vector if k % 3 != 2 else nc.gpsimd
                        eng.tensor_scalar(
                            out=ind[:, k, :], in0=iota_b[:, :],
                            scalar1=locA[:, g0 + k:g0 + k + 1], scalar2=None,
                            op0=mybir.AluOpType.is_equal)

                msgex = mpool.tile([P, Kg, FA], bf16, tag="msgex")
                if "alpha" not in _SKIP:
                    alpha = apool.tile([P, Kg, H], f32, tag="alpha")
                    nc.vector.tensor_tensor(
                        out=alpha[:, :, :], in0=hsa[:, :, F:FA],
                        in1=adw[:, :, 0:H], op=mybir.AluOpType.add)
                    lrl = apool.tile([P, Kg, H], f32, tag="lrl")
                    nc.vector.scalar_tensor_tensor(
                        out=lrl[:, :, :], in0=alpha[:, :, :], scalar=0.2,
                        in1=alpha[:, :, :],
                        op0=mybir.AluOpType.mult, op1=mybir.AluOpType.max)
                    nc.scalar.activation(msgex[:, :, F:FA], lrl[:, :, :],
                                         mybir.ActivationFunctionType.Exp)
                if "msg" not in _SKIP:
                    nc.vector.tensor_tensor(
                        out=msgex[:, :, 0:F].rearrange(
                            "p k (h c) -> p k h c", h=H),
                        in0=hsa[:, :, 0:F].rearrange(
                            "p k (h c) -> p k h c", h=H),
                        in1=msgex[:, :, F:FA].to_broadcast([P, Kg, H, C]),
                        op=mybir.AluOpType.mult)

                for t in tiles:
                    pso = epsum.tile([P, FA], f32, tag="pso")
                    if "pemm" in _SKIP:
                        nc.vector.memset(pso[:, :], 1.0)
                    else:
                        ranges = [chunks_lo[t], chunks_hi[t]]
                        ks = [k for k0, nk in ranges
                              for k in range(k0 - g0, k0 - g0 + nk)]
                        for i, k in enumerate(ks):
                            nc.tensor.matmul(pso[:, :], lhsT=ind[:, k, :],
                                             rhs=msgex[:, k, :],
                                             start=(i == 0),
                                             stop=(i == len(ks) - 1))

                    den = opool.tile([P, H], f32, tag="den")
                    nc.scalar.activation(den[:, :], pso[:, F:FA],
                                         mybir.ActivationFunctionType.Copy,
                                         bias=1e-16)
                    rec = opool.tile([P, H], f32, tag="rec")
                    nc.vector.reciprocal(rec[:, :], den[:, :])
                    on = opool.tile([P, F], f32, tag="on")
                    nc.vector.tensor_tensor(
                        out=on[:, :].rearrange("p (h c) -> p h c", h=H),
                        in0=pso[:, 0:F].rearrange("p (h c) -> p h c", h=H),
                        in1=rec[:, :].to_broadcast([P, H, C]),
                        op=mybir.AluOpType.mult)
                    on2 = opool.tile([P, F], f32, tag="on2")
                    nc.vector.tensor_tensor(out=on2[:, :], in0=on[:, :],
                                            in1=brep_sb[:, :],
                                            op=mybir.AluOpType.add)
                    on3 = opool.tile([P, F], f32, tag="on3")
                    nc.scalar.activation(on3[:, :], on2[:, :],
                                         mybir.ActivationFunctionType.Relu)
                    nc.sync.dma_start(out[t * P:(t + 1) * P, :], on3[:, :])

    nc.finalize()
    return nc


_KERNEL_CACHE = {}


def _get_kernel(plan):
    key = (tuple(plan["K"]), tuple(plan["Klo"]), plan["npad"])
    if key not in _KERNEL_CACHE:
        _KERNEL_CACHE[key] = _build_layer_kernel(plan)
    return _KERNEL_CACHE[key]


def _run_layer(nc, maps, trace=False):
    last = None
    for attempt in range(3):
        try:
            res = run_bass_kernel_spmd(nc, maps, list(range(len(maps))),
                                       trace=trace)
            outs = [r["out"] for r in res.results]
            return np.concatenate(outs, axis=0), res
        except Exception as e:
            last = e
            import time as _time
            _time.sleep(2.0 * (attempt + 1))
    raise last


def kernel(x, edge_index, W1, a_src1, a_dst1, b1, W2, a_src2, a_dst2, b2,
           _trace=False, _collect=None):
    x = np.asarray(x, dtype=np.float32)
    edge_index = np.asarray(edge_index)
    assert x.shape == (N_NODES, F), x.shape
    assert edge_index.shape == (2, N_EDGES), edge_index.shape

    loops = np.arange(N_NODES, dtype=np.int64)
    src = np.concatenate([edge_index[0].astype(np.int64), loops])
    dst = np.concatenate([edge_index[1].astype(np.int64), loops])

    plan = _make_plan(src, dst, N_NODES, N_CORES)
    nc = _get_kernel(plan)

    maps1 = _layer_inputs(plan, x, np.asarray(W1), np.asarray(a_src1),
                          np.asarray(a_dst1), np.asarray(b1))
    o1, res1 = _run_layer(nc, maps1, trace=_trace)

    maps2 = _layer_inputs(plan, o1[: plan["npad"]], np.asarray(W2),
                          np.asarray(a_src2), np.asarray(a_dst2),
                          np.asarray(b2))
    o2, res2 = _run_layer(nc, maps2, trace=_trace)

    if _collect is not None:
        _collect.extend([res1, res2])
    return o2[:N_NODES].astype(np.float32)
